# revision 30
# baseline (speedup 1.0000x reference)
"""Trainium2 Bass kernel for nn_Classifier_48223892799748 (retrieval_knn).

Computes sim = (D + enc_pm @ cent_pm.T) / 2 where
  enc_pm = sign((samples - 0.5) @ weight.T)  in {+1,-1}
  cent_pm = centroids mapped {0,1} -> {-1,+1}

Sharding: data-parallel over the batch dim (8192 -> 1024 rows per core,
8 cores). weight / centroids replicated.

Device layout: everything is computed transposed ([D, B] / [C, B]) so that
the sign-encoded matmul-1 output tile [128 d, 512 b] feeds matmul-2
directly as the moving operand (contraction over d) with no on-device
transpose.

Matmul-1 modes (MM1_MODE env):
  fp8dr (default): samples quantized to fp8e4m3 on host, weights (+/-1,
                   exact in fp8) as fp8; contraction over IN_F=1024 done as
                   4 DoubleRow pairs of 256 at 0.5 cyc/row -> 2x the f32r
                   PE throughput. Host-simulated rel err 0.0089 (gate 2e-2):
                   only the samples see quantization, and a sign flip on
                   ~0.8% of encoding bits perturbs the 10000-bit match count
                   by ~sqrt(85).
  f32r:            single-pass float32r (1 cyc/row). rel err ~8e-4.
  bf16_hilo / fp16 / bf16: earlier experiments, kept for A/B.

Matmul-2 is exact in all modes (+/-1 operands, integer fp32 accum) and runs
as fp8e4m3 DoubleRow. In fp8dr mode the whole PE stream is fp8-DoubleRow,
so matmul-2 interleaves between matmul-1 blocks with no PE dtype-mode
transitions and no serial tail (MM2_PHASE=0). For f32r mode the phase
split (MM2_PHASE=1) avoids 158 f32r<->fp8 mode transitions.
"""

import sys

if "/opt/trn_rl_repo" not in sys.path:
    sys.path.insert(0, "/opt/trn_rl_repo")

import ml_dtypes
import numpy as np

import concourse.bass as bass
import concourse.mybir as mybir
import concourse.tile as tile
from concourse import bacc
from concourse.bass_utils import run_bass_kernel_spmd

# The container's `antenv` package is a stub without `axon_hooks`; if tracing
# is ever requested (BASS_TRACE=1), run_bass_kernel_spmd imports it and would
# crash. Provide a stub module (hook=None -> tracing skipped gracefully)
# unless something (e.g. a test harness) registered a real one already.
try:  # pragma: no cover
    import antenv.axon_hooks  # noqa: F401
except ImportError:
    import types as _types

    import antenv as _antenv

    _hooks = _types.ModuleType("antenv.axon_hooks")
    _hook_store = {"h": None}
    _hooks.set_axon_ntff_profile_hook = lambda h: _hook_store.__setitem__("h", h)
    _hooks.get_axon_ntff_profile_hook = lambda: _hook_store["h"]
    sys.modules["antenv.axon_hooks"] = _hooks
    _antenv.axon_hooks = _hooks

BF16 = ml_dtypes.bfloat16
FP8NP = ml_dtypes.float8_e4m3

B, IN_F, D, C = 8192, 1024, 10000, 100
N_CORES = 8
B_SH = B // N_CORES          # 1024 batch rows per core
KC = IN_F // 128             # 8 contraction chunks for matmul 1
KP = KC // 2                 # 4 DoubleRow contraction pairs (256 each)
DT = (D + 127) // 128        # 79 d-tiles
D_PAD = DT * 128             # 10112
NB = B_SH // 512             # 2 psum-width chunks of the local batch
CENTER = 0.5

# matmul-1 mode: "fp8dr" | "f32r" | "bf16_hilo" | "fp16" | "bf16"
import os as _os
MM1_MODE = _os.environ.get("MM1_MODE", "fp8dr")
USE_F32R = MM1_MODE == "f32r"
USE_FP8DR = MM1_MODE == "fp8dr"
# matmul-2 as a single fp8-DoubleRow phase AFTER all matmul-1 work (needed
# for non-fp8 mm1 streams to avoid per-tile PE dtype-mode transitions);
# default on: interleaving mm2 per-pair into the mm1 stream was measured
# slower (259ns vs 247ns per-matmul cadence) than uniform blocks.
MM2_PHASE = _os.environ.get("MM2_PHASE", "1") == "1"
# flush completed mm2 pairs as a uniform block every MM2_EVERY d-tiles so
# only the final ~MM2_EVERY/2 pairs remain in the serial tail (0 = single
# phase at the end). fp8dr only.
MM2_EVERY = int(_os.environ.get("MM2_EVERY", "20"))
# PE p-state warm-up dummy matmuls (0 = off)
WARM_MM = int(_os.environ.get("WARM_MM", "24"))
NPAIR = (DT + 1) // 2        # 40 d-tile pairs for DoubleRow matmul-2
D_PAD2 = NPAIR * 256         # 10240
C_PAD = 112                  # DoubleRow weight AP needs byte-step %16 == 0

# Stash of the last BassKernelResults (exec_time_ns etc.) for test harnesses.
LAST_RUN = None
_NC_CACHE = None


def _build_nc():
    nc = bacc.Bacc("TRN2", target_bir_lowering=False)
    f32 = mybir.dt.float32
    f32r = mybir.dt.float32r
    bf16 = mybir.dt.bfloat16
    fp16 = mybir.dt.float16
    fp8 = mybir.dt.float8e4
    SIGN = mybir.ActivationFunctionType.Sign
    COPY = mybir.ActivationFunctionType.Copy
    DR = mybir.MatmulPerfMode.DoubleRow

    # DRAM I/O (per-core shard layouts, see host prep in kernel()):
    #   fp8dr path:
    #     sf: [128 k_in, KP, 2, B_SH] fp8   (samples-0.5).T quantized
    #     wt: [DT, 128 k_in, KP, 2, 128 d_in] fp8  weight.T tiles (+/-1)
    #   f32r path:
    #     sf: [128 k_in, KC, B_SH] f32     (samples-0.5).T
    #     wt: [DT, 128 k_in, KC, 128 d_in] f32r  weight.T tiles (+/-1)
    #   bf16 hi/lo path:
    #     sh/sl: [128 k_in, KC, B_SH] bf16 (samples-0.5).T hi/lo
    #     wt:    [DT, 128 k_in, KC, 128 d_in] bf16
    #   ct:  [128 d_in, NPAIR, 2, C_PAD] fp8  centroids.T DoubleRow tiles
    #   out: [C, B_SH] f32                 sim.T shard
    lp = {"fp8dr": fp8, "f32r": f32r, "bf16_hilo": bf16, "fp16": fp16,
          "bf16": bf16}[MM1_MODE]
    if USE_FP8DR:
        sf_d = nc.dram_tensor("sf", [128, KP, 2, B_SH], fp8, kind="ExternalInput")
        wt_d = nc.dram_tensor("wt", [DT, 128, KP, 2, 128], fp8, kind="ExternalInput")
    elif USE_F32R:
        sf_d = nc.dram_tensor("sf", [128, KC, B_SH], f32, kind="ExternalInput")
        wt_d = nc.dram_tensor("wt", [DT, 128, KC, 128], f32r, kind="ExternalInput")
    elif MM1_MODE == "bf16_hilo":
        sh_d = nc.dram_tensor("sh", [128, KC, B_SH], bf16, kind="ExternalInput")
        sl_d = nc.dram_tensor("sl", [128, KC, B_SH], bf16, kind="ExternalInput")
        wt_d = nc.dram_tensor("wt", [DT, 128, KC, 128], bf16, kind="ExternalInput")
    else:
        sh_d = nc.dram_tensor("sh", [128, KC, B_SH], lp, kind="ExternalInput")
        wt_d = nc.dram_tensor("wt", [DT, 128, KC, 128], lp, kind="ExternalInput")
    ct_d = nc.dram_tensor("ct", [128, NPAIR, 2, C_PAD], fp8, kind="ExternalInput")
    out_d = nc.dram_tensor("out", [C, B_SH], f32, kind="ExternalOutput")

    w_dt = lp

    with tile.TileContext(nc) as tc:
        with (
            tc.tile_pool(name="const", bufs=1) as const_pool,
            tc.tile_pool(name="wts", bufs=6) as w_pool,
            tc.tile_pool(name="enc", bufs=3) as enc_pool,
            tc.tile_pool(name="outp", bufs=1) as out_pool,
            tc.tile_pool(name="ps1", bufs=3, space=bass.MemorySpace.PSUM) as ps1_pool,
            tc.tile_pool(name="ps2", bufs=1, space=bass.MemorySpace.PSUM) as ps2_pool,
        ):
            preamble_rest = None
            deferred_dma = {}
            ps2 = [
                ps2_pool.tile([C_PAD, 512], mybir.dt.float32, tag=f"ps2_{b}", name=f"ps2_{b}")
                for b in range(NB)
            ]
            if USE_FP8DR and WARM_MM:
                # p-state warm-up: the PE clock ramps to 2.4 GHz only after
                # ~3us of continuous SWITCHING activity; the first real
                # operands land at ~10.8us while the PE exits init at ~6.5us.
                # All-ones dummy DR matmuls (zeros create no activity and do
                # not ramp the clock -- measured) bridge that window so the
                # real stream starts at full clock. They write ps2[:, :128]
                # with start=True; the first real mm2 also uses start=True,
                # which resets the bank.
                warm_s = const_pool.tile([128, 2, 128], fp8)
                warm_w = const_pool.tile([128, 2, C_PAD], fp8)
                nc.gpsimd.memset(warm_s[:], 1.0)
                nc.gpsimd.memset(warm_w[:], 1.0)
                for k in range(WARM_MM):
                    nc.tensor.matmul(
                        ps2[k % NB][:, 0:128],
                        warm_w[:],
                        warm_s[:],
                        start=True,
                        stop=True,
                        perf_mode=DR,
                        skip_group_check=True,
                    )


            if USE_FP8DR:
                s8 = const_pool.tile([128, KP, 2, B_SH], fp8)
                # only the kp=0 pair is loaded before the d-loop, split by
                # b-chunk so the very first matmul waits on 128 KB; the
                # remaining sample pairs and the centroids are staggered
                # across early d-iterations so they don't starve the weight
                # tile stream (observed 2-3us PE stalls when issued in bulk).
                # Sample loads issue from the (otherwise idle) GpSimd engine
                # so they don't serialize behind weight-tile issues on Sync:
                # each engine's DMA issue takes ~0.7us, and the first matmul
                # sits behind every issue queued before its data on the same
                # engine. (j, b)-split spreads descriptor chains over queues.
                for b in range(NB):
                    for j in range(2):
                        nc.gpsimd.dma_start(
                            s8[:, 0, j, bass.ts(b, 512)],
                            sf_d[:, 0, j, bass.ts(b, 512)],
                        )

                def _load_s8_rest():
                    # dt=0's matmuls contract over ALL kp pairs, so every
                    # sample chunk must be written (in program order) before
                    # the dt=0 matmul block; j-split spreads queues
                    for kp in range(1, KP):
                        for j in range(2):
                            nc.gpsimd.dma_start(s8[:, kp, j, :], sf_d[:, kp, j, :])

                deferred_dma[0] = _load_s8_rest

                # tiny duplicate of the first weight pair so the very first
                # matmul waits on 32 KB, not the full 128 KB w[0] load
                w00 = const_pool.tile([128, 2, 128], fp8)
                nc.sync.dma_start(w00[:], wt_d[0, :, 0, :, :])
                s_streams = [s8]
            elif USE_F32R:
                s_f = const_pool.tile([128, KC, B_SH], f32)
                s_r = const_pool.tile([128, KC, B_SH], f32r)
                # per-kc loads + f32->f32r rounding casts (DVE is otherwise
                # idle); split so PE can start after the first chunk.
                for b in range(NB):
                    nc.sync.dma_start(
                        s_f[:, 0, bass.ts(b, 512)], sf_d[:, 0, bass.ts(b, 512)]
                    )
                    nc.vector.tensor_copy(
                        s_r[:, 0, bass.ts(b, 512)], s_f[:, 0, bass.ts(b, 512)]
                    )

                def preamble_rest():
                    for kc in range(1, KC):
                        nc.sync.dma_start(s_f[:, kc, :], sf_d[:, kc, :])
                        nc.vector.tensor_copy(s_r[:, kc, :], s_f[:, kc, :])

                w00 = const_pool.tile([128, 128], f32r)
                nc.sync.dma_start(w00[:], wt_d[0, :, 0, :])
                s_streams = [s_r]
            elif MM1_MODE == "bf16_hilo":
                s_hi = const_pool.tile([128, KC, B_SH], bf16)
                s_lo = const_pool.tile([128, KC, B_SH], bf16)
                for kc in range(KC):
                    nc.sync.dma_start(s_hi[:, kc, :], sh_d[:, kc, :])
                    nc.sync.dma_start(s_lo[:, kc, :], sl_d[:, kc, :])
                s_streams = [s_hi, s_lo]
            else:
                s_hi = const_pool.tile([128, KC, B_SH], lp)
                for kc in range(KC):
                    nc.sync.dma_start(s_hi[:, kc, :], sh_d[:, kc, :])
                s_streams = [s_hi]

            cent = const_pool.tile([128, NPAIR, 2, C_PAD], fp8)
            if USE_FP8DR:
                # centroids aren't needed until the first mm2 flush (dt=19);
                # defer well past the sample-load burst so the weight tile
                # stream isn't starved, and split across two queues
                def _load_cent():
                    h = NPAIR // 2
                    nc.gpsimd.dma_start(cent[:, :h], ct_d[:, :h])
                    nc.gpsimd.dma_start(cent[:, h:], ct_d[:, h:])

                # must be issued (program order) before the first mm2 flush
                # at dt = MM2_EVERY-1 reads cent, with a few dt of transfer
                # headroom for the 1.15 MB
                cent_dt = max(0, min(12, (MM2_EVERY or DT) - 6))
                deferred_dma[cent_dt] = _load_cent
            if MM2_PHASE:
                # all sign-encodings buffered on-chip; matmul-2 runs as
                # uniform fp8-DoubleRow blocks (chunked per MM2_EVERY)
                enc_all = const_pool.tile([128, NPAIR, 2, B_SH], fp8)
                # phantom j=1 half of the final pair (dt=79 doesn't exist):
                # zero it so 0-weight x garbage(NaN) can't poison the PSUM.
                # On Vector: a 1us memset on GpSimd would delay the sample
                # DMA issues queued behind it.
                nc.vector.memset(enc_all[:, NPAIR - 1, 1, :], 0.0)


            # software pipeline: matmul2 for pair p is issued on PE after
            # the matmul1 block of pair p+1, so PE never waits on the Sign
            # activation round-trip.
            pending = []
            next_pair = 0

            def flush_pending():
                t0, encs = pending.pop(0)
                for b in range(NB):
                    nc.tensor.matmul(
                        ps2[b][:],
                        cent[:, t0, :, :],
                        encs[b][:],
                        start=(t0 == 0),
                        stop=(t0 == NPAIR - 1),
                        perf_mode=DR,
                    )

            n_acc = len(s_streams) * (KP if USE_FP8DR else KC)
            for dt in range(DT):
                if USE_FP8DR:
                    w = w_pool.tile([128, KP, 2, 128], w_dt, tag="w", name=f"w_{dt}")
                    # two half-tile DMAs -> two queues deliver each weight
                    # tile in parallel (sample/centroid issues live on the
                    # GpSimd queue, so Sync only ever issues weights + out).
                    # For the first few tiles the second half issues from
                    # Scalar (idle until its first Sign at ~12.5us): one
                    # issue slot (~0.65us) per engine per tile halves the
                    # ramp-phase issue latency and removes the dt<8 PE
                    # starvation.
                    h2 = nc.scalar if dt < 4 else nc.sync
                    nc.sync.dma_start(w[:, : KP // 2, :, :], wt_d[dt, :, : KP // 2, :, :])
                    h2.dma_start(w[:, KP // 2 :, :, :], wt_d[dt, :, KP // 2 :, :, :])
                else:
                    w = w_pool.tile([128, KC, 128], w_dt, tag="w", name=f"w_{dt}")
                    nc.sync.dma_start(w[:, : KC // 2, :], wt_d[dt, :, : KC // 2, :])
                    nc.sync.dma_start(w[:, KC // 2 :, :], wt_d[dt, :, KC // 2 :, :])
                if dt in deferred_dma:
                    deferred_dma[dt]()
                if dt == 0 and not USE_FP8DR:
                    # deferred preamble: remaining sample chunks + centroids
                    if preamble_rest is not None:
                        preamble_rest()
                    nc.sync.dma_start(cent[:], ct_d[:])
                ps1 = [
                    ps1_pool.tile(
                        [128, 512], mybir.dt.float32, tag=f"ps1_{b}", name=f"ps1_{dt}_{b}"
                    )
                    for b in range(NB)
                ]
                acc = 0
                if USE_FP8DR:
                    for kp in range(KP):
                        w_src = w00 if (dt == 0 and kp == 0) else w[:, kp, :, :]
                        for b in range(NB):
                            nc.tensor.matmul(
                                ps1[b][:],
                                w_src,
                                s8[:, kp, :, bass.ts(b, 512)],
                                start=(acc == 0),
                                stop=(acc == n_acc - 1),
                                perf_mode=DR,
                            )
                        acc += 1
                else:
                    for kc in range(KC):
                        w_src = w00 if (USE_F32R and dt == 0 and kc == 0) else w[:, kc, :]
                        for s_t in s_streams:
                            for b in range(NB):
                                nc.tensor.matmul(
                                    ps1[b][:],
                                    w_src,
                                    s_t[:, kc, bass.ts(b, 512)],
                                    start=(acc == 0),
                                    stop=(acc == n_acc - 1),
                                )
                            acc += 1
                if MM2_PHASE:
                    for b in range(NB):
                        nc.scalar.activation(
                            enc_all[:, dt // 2, dt % 2, bass.ts(b, 512)],
                            ps1[b][:],
                            SIGN,
                        )
                    if (
                        USE_FP8DR
                        and MM2_EVERY
                        and (dt + 1) % MM2_EVERY == 0
                        and dt != DT - 1
                    ):
                        # mid-stream uniform mm2 block over the pairs whose
                        # sign-encodings are already complete
                        flush_to = (dt + 1) // 2
                        for t in range(next_pair, flush_to):
                            for b in range(NB):
                                nc.tensor.matmul(
                                    ps2[b][:],
                                    cent[:, t, :, :],
                                    enc_all[:, t, :, bass.ts(b, 512)],
                                    start=(t == 0),
                                    stop=False,
                                    perf_mode=DR,
                                )
                        next_pair = flush_to
                else:
                    j = dt % 2
                    if j == 0:
                        cur_pair = [
                            enc_pool.tile(
                                [128, 2, 512], fp8, tag=f"enc_{b}", name=f"e_{dt}_{b}"
                            )
                            for b in range(NB)
                        ]
                    for b in range(NB):
                        nc.scalar.activation(cur_pair[b][:, j, :], ps1[b][:], SIGN)
                    if dt == DT - 1 and j == 0:
                        # odd tile of the final pair does not exist: zero it so
                        # 0-weight x garbage(NaN) cannot poison the PSUM
                        for b in range(NB):
                            nc.gpsimd.memset(cur_pair[b][:, 1, :], 0.0)
                    if j == 1 or dt == DT - 1:
                        pending.append((dt // 2, cur_pair))
                    if len(pending) >= 2:
                        flush_pending()
            while pending:
                flush_pending()
            if MM2_PHASE:
                # final mm2 chunk b-major: ps2[0] completes first so its
                # output activation + DMA overlap ps2[1]'s matmuls
                for b in range(NB):
                    for t in range(next_pair, NPAIR):
                        nc.tensor.matmul(
                            ps2[b][:],
                            cent[:, t, :, :],
                            enc_all[:, t, :, bass.ts(b, 512)],
                            start=(t == 0),
                            stop=(t == NPAIR - 1),
                            perf_mode=DR,
                        )
                    ob = out_pool.tile(
                        [C, 512], mybir.dt.float32, tag=f"ob_{b}", name=f"ob_{b}"
                    )
                    nc.scalar.activation(ob[:], ps2[b][:C, :], COPY, bias=D / 2.0, scale=0.5)
                    nc.sync.dma_start(out_d[:, bass.ts(b, 512)], ob[:])
            else:
                for b in range(NB):
                    ob = out_pool.tile(
                        [C, 512], mybir.dt.float32, tag=f"ob_{b}", name=f"ob_{b}"
                    )
                    nc.scalar.activation(ob[:], ps2[b][:C, :], COPY, bias=D / 2.0, scale=0.5)
                    nc.sync.dma_start(out_d[:, bass.ts(b, 512)], ob[:])

    nc.compile()
    return nc


def _get_nc():
    global _NC_CACHE
    if _NC_CACHE is None:
        _NC_CACHE = _build_nc()
    return _NC_CACHE


def kernel(samples, weight, centroids):
    global LAST_RUN
    samples = np.asarray(samples, dtype=np.float32)
    weight = np.asarray(weight, dtype=np.float32)
    centroids = np.asarray(centroids)

    # ---- host-side marshalling (layout + dtype only) ----
    # centered samples, transposed to [IN_F, B]
    scT = (samples - np.float32(CENTER)).T

    def s_core(a, c):
        # [IN_F, B_SH] -> [128 k_in, KC, B_SH]
        blk = a[:, c * B_SH : (c + 1) * B_SH]
        return np.ascontiguousarray(blk.reshape(KC, 128, B_SH).transpose(1, 0, 2))

    FP16 = np.float16
    w_np = {"fp8dr": FP8NP, "f32r": np.float32, "bf16_hilo": BF16, "fp16": FP16,
            "bf16": BF16}[MM1_MODE]
    # weight.T tiles: wt[dt, k_in, kc, d_in] = weight[dt*128+d_in, kc*128+k_in]
    wpad = np.zeros((D_PAD, IN_F), dtype=w_np)
    wpad[:D] = weight.astype(w_np)  # +/-1, exact in bf16/f32r/fp8
    wt = np.ascontiguousarray(wpad.reshape(DT, 128, KC, 128).transpose(0, 3, 2, 1))
    if USE_FP8DR:
        wt = wt.reshape(DT, 128, KP, 2, 128)

    # DoubleRow centroid tiles: ct[d_in, t, j, c] = cent_pm[c, t*256+j*128+d_in]
    cpad = np.zeros((D_PAD2, C_PAD), dtype=np.float32)
    cpad[:D, :C] = np.where(centroids, np.float32(1.0), np.float32(-1.0)).T
    ct = np.ascontiguousarray(
        cpad.reshape(NPAIR, 2, 128, C_PAD).transpose(2, 0, 1, 3).astype(FP8NP)
    )

    if USE_FP8DR:
        s_q = scT.astype(FP8NP)
        in_maps = [
            {"sf": s_core(s_q, c).reshape(128, KP, 2, B_SH), "wt": wt, "ct": ct}
            for c in range(N_CORES)
        ]
    elif USE_F32R:
        in_maps = [
            {"sf": s_core(scT, c), "wt": wt, "ct": ct} for c in range(N_CORES)
        ]
    elif MM1_MODE == "bf16_hilo":
        s_hi = scT.astype(BF16)
        s_lo = (scT - s_hi.astype(np.float32)).astype(BF16)
        in_maps = [
            {"sh": s_core(s_hi, c), "sl": s_core(s_lo, c), "wt": wt, "ct": ct}
            for c in range(N_CORES)
        ]
    else:
        s_hi = scT.astype(w_np)
        in_maps = [
            {"sh": s_core(s_hi, c), "wt": wt, "ct": ct} for c in range(N_CORES)
        ]

    nc = _get_nc()
    res = run_bass_kernel_spmd(nc, in_maps, core_ids=list(range(N_CORES)))
    LAST_RUN = res

    # gather: out[c] is sim.T for batch rows [c*B_SH, (c+1)*B_SH)
    return np.vstack(
        [np.asarray(res.results[c]["out"]).T for c in range(N_CORES)]
    ).astype(np.float32)
